# revision 28
# baseline (speedup 1.0000x reference)
"""Llama attention layer (B=2, S=2048, D=2048, H=16, HD=128, RoPE, causal)
on 8 Trainium2 NeuronCores.

Sharding: core c -> (batch b = c//4, head group g = c%4 of 4 heads).
Each core computes q/k/v projections for its 512 columns of wq/wk/wv,
RoPE, causal attention for its 4 heads, and the out-projection against
its 512 rows of wo (a partial sum over head groups). The host sums the
4 partials per batch and stacks the 2 batches.

All device matmuls run in bf16 with fp32 PSUM accumulation. Softmax is
computed without max-subtraction (scores here are bounded ~|9|), with
the denominator obtained from an M=1 ones-matmul over exp(scores^T).

Perf structure (vs the naive version):
- xT is packed m-major on the host so each 128-column block of x^T is
  one contiguous DMA; the V-projection m-groups start as soon as their
  own block lands instead of waiting for all of xT.
- Diagonal 128x512 score/exp/attn-V work is narrowed to the causally
  valid query columns; the V-matmul is split per 128-col region so each
  region's accumulation group can close with its own stop flag.
- Softmax denominator: DVE adds exp-chunk pairs, one ones-matmul per
  pair accumulates into PSUM. (Deeper reduction trees on GpSimd were
  tried and regress: the PE executes in program order, so a ones-matmul
  that waits on a ~1.2us GpSimd add stalls every later matmul.)
- Output partials are written bf16 (host accumulates in fp32), halving
  output DMA.
"""

import os
import sys

import numpy as np
import ml_dtypes

if "/opt/trn_rl_repo" not in sys.path:
    sys.path.insert(0, "/opt/trn_rl_repo")

import concourse.bass as bass  # noqa: E402
import concourse.mybir as mybir  # noqa: E402
import concourse.bacc as bacc  # noqa: E402
import concourse.tile as tile  # noqa: E402

BF16 = ml_dtypes.bfloat16

B, S, D, H = 2, 2048, 2048, 16
HD = D // H            # 128, head dim
G = 4                  # head groups (cores per batch)
NH = H // G            # 4 heads per core
DG = NH * HD           # 512, per-core head width
P = 128
KO = D // P            # 16 k-subtiles over D
NKT = S // P           # 16 key chunks of 128
NQT = S // 512         # 4 q tiles of 512
QT = 512
ROPE_THETA = 10000.0
SCALE = 1.0 / float(np.sqrt(HD))

N_CORES = 8

_BUILT = None  # (nc,) cache


def build_module():
    fp32 = mybir.dt.float32
    bf16 = mybir.dt.bfloat16

    nc = bacc.Bacc("TRN2", target_bir_lowering=False, debug=False,
                   num_devices=N_CORES, num_swdge_queues=4)

    xT = nc.dram_tensor("xT", [P, NKT, KO, P], bf16, kind="ExternalInput")
    wq = nc.dram_tensor("wq", [P, KO, DG], bf16, kind="ExternalInput")
    wk = nc.dram_tensor("wk", [P, KO, DG], bf16, kind="ExternalInput")
    wv = nc.dram_tensor("wv", [P, KO, DG], bf16, kind="ExternalInput")
    wo = nc.dram_tensor("wo", [P, NH, D], bf16, kind="ExternalInput")
    cosT = nc.dram_tensor("cosT", [P, S], bf16, kind="ExternalInput")
    sinT = nc.dram_tensor("sinT", [P, S], bf16, kind="ExternalInput")
    maskT = nc.dram_tensor("maskT", [P, P], bf16, kind="ExternalInput")
    out = nc.dram_tensor("out", [P, NKT, D], bf16, kind="ExternalOutput")

    Exp = mybir.ActivationFunctionType.Exp

    with tile.TileContext(nc) as tc:
        with tc.tile_pool(name="const", bufs=1) as const, \
             tc.tile_pool(name="big", bufs=1) as big:
            ones = const.tile([P, P], bf16)
            nc.vector.memset(ones, 1.0)
            # dummy exp so the ACT Exp table loads during the DMA prefix,
            # not at the first real exp in the attention phase
            warm = const.tile([1, 1], fp32)
            nc.scalar.activation(warm, ones[0:1, 0:1],
                                 mybir.ActivationFunctionType.Exp)
            warm_mm = const.tile([P, QT], bf16)
            nc.vector.memset(warm_mm, 0.0)

            qT_sb = big.tile([P, NH, S], bf16)   # per head: [HD, S]
            kT_sb = big.tile([P, NH, S], bf16)
            v_sb = big.tile([P, NKT, DG], bf16)  # [key%128, keychunk, dg]
            wo_sb = big.tile([P, NH, D], bf16)
            mask_sb = const.tile([P, P], bf16)
            ao0_sb = big.tile([P, QT], bf16)     # (qt=0, h=0) attention out

            def attend(qt, h, pool_s, s_bufs, pool_o, pool_sum, pool_ax,
                       ax_bufs, pool_axp, axp_bufs, pool_axq, pool_ep,
                       ep_bufs, dst):
                """Causal attention for one (q-tile, head) into dst."""
                n_kt = 4 * (qt + 1)  # causal: key chunks 0..n_kt-1
                ps_o = pool_o.tile([P, QT], fp32, tag="ps_o")
                # all-ones [128,128] lhsT -> every psum row holds sumexp:
                # no partition-broadcast needed later
                ps_sum = pool_sum.tile([P, QT], fp32, tag="ps_sum")
                # denominator plan: early chunk pairs are added into quads
                # on DVE and their ones-matmul is DEFERRED 4 chunks (so the
                # PE, which executes in program order, never waits on the
                # DVE adds); the last 4 chunks use immediate pair matmuls.
                n_equads = max(0, n_kt - 4) // 4
                mm_total = n_equads + 2
                mm_i = 0
                pending = {}
                ax_prev = None
                pair_prev = None
                for m in range(n_kt):
                    if m in pending:
                        nc.tensor.matmul(ps_sum, ones, pending.pop(m),
                                         start=(mm_i == 0),
                                         stop=(mm_i == mm_total - 1))
                        mm_i += 1
                    # diagonal chunks only cover queries >= their first key:
                    # narrow to columns [colo:QT)
                    o = m - qt * 4
                    colo = max(0, o) * P
                    ps_s = pool_s.tile([P, QT], fp32, tag="ps_s",
                                       bufs=s_bufs)
                    nc.tensor.matmul(ps_s[:, colo:],
                                     kT_sb[:, h, m * P:(m + 1) * P],
                                     qT_sb[:, h,
                                           qt * QT + colo:(qt + 1) * QT],
                                     start=True, stop=True)
                    ax = pool_ax.tile([P, QT], bf16, tag="ax", bufs=ax_bufs)
                    if colo:
                        # zero the causally-dead prefix so the denominator
                        # adds see zeros there
                        nc.gpsimd.memset(ax[:, 0:colo], 0.0)
                    nc.scalar.activation(ax[:, colo:], ps_s[:, colo:],
                                         Exp, scale=SCALE)
                    if o >= 0:
                        # triangular mask on the 128 cols that straddle the
                        # diagonal; later cols are fully valid
                        nc.vector.tensor_mul(ax[:, colo:colo + P],
                                             ax[:, colo:colo + P], mask_sb)
                    # attn @ V, narrowed. Each 128-col region's last
                    # contribution is the diagonal chunk o = region index,
                    # so that slice carries stop=True while the rest keeps
                    # accumulating.
                    vsl = v_sb[:, m, h * HD:(h + 1) * HD]
                    if o < 0:
                        nc.tensor.matmul(ps_o, vsl, ax,
                                         start=(m == 0), stop=False)
                    else:
                        nc.tensor.matmul(ps_o[:, colo:colo + P], vsl,
                                         ax[:, colo:colo + P],
                                         start=(m == 0), stop=True)
                        if colo + P < QT:
                            nc.tensor.matmul(ps_o[:, colo + P:], vsl,
                                             ax[:, colo + P:],
                                             start=(m == 0), stop=False)
                    if m % 2 == 0:
                        ax_prev = ax
                    else:
                        pair = pool_axp.tile([P, QT], bf16, tag="axp",
                                             bufs=axp_bufs)
                        nc.vector.tensor_add(pair, ax_prev, ax)
                        if m >= n_kt - 4:
                            nc.tensor.matmul(ps_sum, ones, pair,
                                             start=(mm_i == 0),
                                             stop=(mm_i == mm_total - 1))
                            mm_i += 1
                        elif m % 4 == 1:
                            pair_prev = pair
                        else:
                            quad = pool_axq.tile([P, QT], bf16, tag="axq",
                                                 bufs=3)
                            nc.vector.tensor_add(quad, pair_prev, pair)
                            pending[min(m + 4, n_kt - 1)] = quad
                rec = pool_ep.tile([P, QT], fp32, tag="rec", bufs=ep_bufs)
                nc.vector.reciprocal_approx_fast(rec, ps_sum)
                nc.vector.tensor_mul(dst, ps_o, rec)

            # ---------------- phase 1: projections + RoPE ----------------
            with tc.tile_pool(name="w_pool", bufs=1) as w_pool, \
                 tc.tile_pool(name="rope", bufs=4) as rope, \
                 tc.tile_pool(name="ps1", bufs=1, space="PSUM") as ps1:
                # DMA order matters: wv first (V-loop gate), then xT in
                # m-major column blocks (the dram layout is packed so block m
                # is contiguous) so V m-group m only waits for its own block,
                # then the K/Q-phase tensors, then phase-2/3 tensors.
                wv_sb = w_pool.tile([P, KO, DG], bf16)
                # m-major like the DRAM packing: per-block DMA is contiguous
                # (4KB/partition). A k-major SBUF layout would make the block
                # DMA scatter 256B lines, which runs ~7x slower.
                xT_sb = w_pool.tile([P, NKT, KO, P], bf16)
                wk_sb = w_pool.tile([P, KO, DG], bf16)
                cos_sb = w_pool.tile([P, S], bf16)
                sin_sb = w_pool.tile([P, S], bf16)
                wq_sb = w_pool.tile([P, KO, DG], bf16)
                # Descriptor generation costs ~0.6us of sequencer time per
                # dma_start, so split the input stream over BOTH hardware DGE
                # queues: the weight stream on the Activation queue (idle in
                # phase 1), the xT column blocks on the SP queue. The V-loop
                # m-group m then only waits for wv (k-sliced, so its first
                # matmuls start early) and its own xT block.
                for ks_ in range(0, KO, 4):
                    nc.scalar.dma_start(wv_sb[:, ks_:ks_ + 4, :],
                                        wv.ap()[:, ks_:ks_ + 4, :])
                for m in range(NKT):
                    nc.sync.dma_start(xT_sb[:, m], xT.ap()[:, m])
                nc.scalar.dma_start(wk_sb, wk.ap())
                nc.scalar.dma_start(cos_sb, cosT.ap())
                nc.scalar.dma_start(sin_sb, sinT.ap())
                nc.scalar.dma_start(wq_sb, wq.ap())
                nc.sync.dma_start(mask_sb, maskT.ap())
                nc.sync.dma_start(wo_sb, wo.ap())

                # PE p-state warmup: the tensor engine clocks up only after
                # ~3us of continuous execution, so chew on zeros while the
                # first wv/xT DMAs land -- the first real matmuls then run at
                # full clock instead of half.
                ps_w = ps1.tile([P, QT], fp32, tag="psv", bufs=4)
                for r in range(5):
                    nc.tensor.matmul(ps_w, ones, warm_mm,
                                     start=(r == 0), stop=(r == 4))

                # V: [keys, dg] natural layout, keychunk tiles of 128.
                # k-OUTER waves of 4 m-groups (4 psum banks): each wv k-slice
                # unlocks 4 matmuls, so the PE ramps as the k-sliced wv DMA
                # trickles in instead of waiting for all of wv.
                for mw in range(0, NKT, 4):
                    pss = [ps1.tile([P, DG], fp32, tag="psv",
                                    name=f"psv{i}", bufs=4) for i in range(4)]
                    for k in range(KO):
                        for i in range(4):
                            nc.tensor.matmul(pss[i], xT_sb[:, mw + i, k, :],
                                             wv_sb[:, k, :],
                                             start=(k == 0),
                                             stop=(k == KO - 1))
                    for i in range(4):
                        nc.vector.tensor_copy(v_sb[:, mw + i, :], pss[i])

                # K then Q: [HD, S] transposed layout + RoPE.
                # Heads processed in pairs so the two psum tags can be
                # double-buffered (2 tags x 2 bufs) -- RoPE of one pair
                # overlaps the matmuls of the next.
                for which, w_sb, dstT in (("k", wk_sb, kT_sb), ("q", wq_sb, qT_sb)):
                    for nt2 in range(2 * NQT):
                        nt, hp = divmod(nt2, 2)
                        sl = slice(nt * QT, (nt + 1) * QT)
                        heads = (2 * hp, 2 * hp + 1)
                        # share the "psv" tag (banks 0-3) so all of phase 1
                        # stays within 4 psum banks, leaving 4-7 free for
                        # the attention pools to start without bank conflicts
                        pss = {}
                        for h in heads:
                            pss[h] = ps1.tile([P, QT], fp32, tag="psv",
                                              name=f"psp{h}", bufs=4)
                        # rhs: the nt-th 512 queries = xT m-blocks 4nt..4nt+3
                        # at fixed k -- a strided [128, 4, 128] AP
                        for k in range(KO):
                            for h in heads:
                                nc.tensor.matmul(
                                    pss[h], w_sb[:, k, h * HD:(h + 1) * HD],
                                    xT_sb[:, nt * 4:(nt + 1) * 4, k, :],
                                    start=(k == 0), stop=(k == KO - 1))
                        for h in heads:
                            ps = pss[h]
                            dst = dstT[:, h, sl]
                            # rope: dst = ps * cos + swap(ps) * sin_signed.
                            # The swapped reads must come from PSUM (the SB-SB
                            # same-base-partition rule forbids them on SBUF);
                            # the straight read goes via a parallel ACT copy so
                            # the psum bank drains fast.
                            tmp = rope.tile([P, QT], bf16, tag="tmp")
                            nc.vector.tensor_mul(tmp[0:64], ps[64:128],
                                                 sin_sb[0:64, sl])
                            nc.vector.tensor_mul(tmp[64:128], ps[0:64],
                                                 sin_sb[64:128, sl])
                            qb = rope.tile([P, QT], bf16, tag="qb")
                            nc.scalar.copy(qb, ps)
                            nc.vector.tensor_mul(dst, qb, cos_sb[:, sl])
                            nc.vector.tensor_add(dst, dst, tmp)

                # attention for (qt=0, h=0) right here: its inputs (kT/qT
                # tile 0, v) are long ready, and PSUM banks 4-7 are free
                # (phase 1 keeps to 4 banks via the shared psv tag). The PE
                # chews on these chunks while phase-1 PSUM drains and the
                # phase-2 pools open, bridging the transition gap.
                with tc.tile_pool(name="a0ps", bufs=1, space="PSUM") as a0ps, \
                     tc.tile_pool(name="a0sb", bufs=1) as a0sb:
                    attend(0, 0, a0ps, 2, a0ps, a0ps, a0sb, 4, a0sb, 2,
                           a0sb, a0sb, 1, ao0_sb)

            # ---------------- phases 2+3 ----------------
            with tc.tile_pool(name="big2", bufs=1) as big2:
                aoT_sb = big2.tile([P, NH, S], bf16)  # attention out^T

                # phases 2+3 interleaved: attention for q-tile qt, then the
                # out-projection rows it unblocks (their matmuls have no ACT
                # dependency and fill the exp-latency bubbles)
                with tc.tile_pool(name="ax_pool", bufs=20) as ax_pool, \
                     tc.tile_pool(name="axp_pool", bufs=6) as axp_pool, \
                     tc.tile_pool(name="axq_pool", bufs=3) as axq_pool, \
                     tc.tile_pool(name="ep", bufs=4) as ep, \
                     tc.tile_pool(name="stage", bufs=6) as stage, \
                     tc.tile_pool(name="ps2s", bufs=3, space="PSUM") as ps2s, \
                     tc.tile_pool(name="ps2o", bufs=2, space="PSUM") as ps2o, \
                     tc.tile_pool(name="ps2", bufs=1, space="PSUM") as ps2, \
                     tc.tile_pool(name="ps3", bufs=2, space="PSUM") as ps3:
                    def emit_outproj(qo):
                        for n in range(D // QT):
                            nsl = slice(n * QT, (n + 1) * QT)
                            ps = ps3.tile([P, QT], fp32, tag="ps_out")
                            for h in range(NH):
                                if h == 0 and qo < 4:
                                    lhs = ao0_sb[:, qo * P:(qo + 1) * P]
                                else:
                                    lhs = aoT_sb[:, h, qo * P:(qo + 1) * P]
                                nc.tensor.matmul(
                                    ps, lhs, wo_sb[:, h, nsl],
                                    start=(h == 0), stop=(h == NH - 1))
                            ob = stage.tile([P, QT], bf16, tag="ob")
                            nc.vector.tensor_copy(ob, ps)
                            nc.sync.dma_start(out.ap()[:, qo, nsl], ob)

                    for qt in range(NQT):
                        qsl = slice(qt * QT, (qt + 1) * QT)
                        for h in range(NH):
                            if qt == 0 and h == 0:
                                continue  # computed at the end of phase 1
                            attend(qt, h, ps2s, 3, ps2o, ps2, ax_pool, 20,
                                   axp_pool, 6, axq_pool, ep, 4,
                                   aoT_sb[:, h, qsl])

                            # out-projection for the previous q-tile's rows,
                            # interleaved between this tile's heads so the
                            # normalize latency of the previous tile's last
                            # head hides under this head's chunk matmuls
                            if qt > 0:
                                emit_outproj(4 * (qt - 1) + h)

                    # the last q-tile's rows have nothing to hide behind
                    for qo in range(4 * (NQT - 1), 4 * NQT):
                        emit_outproj(qo)

    nc.compile()
    return nc


def _rope_tables():
    inv_freq = 1.0 / (ROPE_THETA ** (np.arange(0, HD, 2, dtype=np.float64) / HD))
    pos = np.arange(S, dtype=np.float64)
    freqs = np.outer(pos, inv_freq)                    # [S, HD/2]
    emb = np.concatenate([freqs, freqs], axis=-1)      # [S, HD]
    cos = np.cos(emb).T.astype(BF16)                   # [HD, S]
    sin = np.sin(emb).T.astype(np.float32)
    sin[: HD // 2] *= -1.0                             # fold rotate_half sign
    return cos, sin.astype(BF16)


def _pack_kd(a):
    """[D, N] -> [P, D//P, N] with d = ko*P + p."""
    d, n = a.shape
    return np.ascontiguousarray(
        a.reshape(d // P, P, n).transpose(1, 0, 2)).astype(BF16)


def _pack_xT(xb):
    """x[b] [S, D] -> [P, NKT, KO, P] m-major so each 128-col block of x^T
    is one contiguous DMA."""
    t = _pack_kd(np.ascontiguousarray(xb.T))           # [P, KO, S]
    return np.ascontiguousarray(
        t.reshape(P, KO, NKT, P).transpose(0, 2, 1, 3))


def make_in_maps(x, wq, wk, wv, wo):
    cosT, sinT = _rope_tables()
    i = np.arange(P)[:, None]
    j = np.arange(P)[None, :]
    mask = (i <= j).astype(BF16)

    xT_packed = [_pack_xT(x[b]) for b in range(B)]
    in_maps = []
    for c in range(N_CORES):
        b, g = divmod(c, G)
        gsl = slice(g * DG, (g + 1) * DG)
        in_maps.append({
            "xT": xT_packed[b],
            "wq": _pack_kd(wq[:, gsl]),
            "wk": _pack_kd(wk[:, gsl]),
            "wv": _pack_kd(wv[:, gsl]),
            "wo": _pack_kd(np.ascontiguousarray(wo[gsl, :])),
            "cosT": cosT,
            "sinT": sinT,
            "maskT": mask,
        })
    return in_maps


def assemble_output(results):
    """results: list of 8 dicts with 'out' [P, NKT, D] bf16 partials."""
    full = np.empty((B, S, D), dtype=np.float32)
    for b in range(B):
        acc = None
        for g in range(G):
            r = results[b * G + g]["out"].astype(np.float32)
            part = r.transpose(1, 0, 2).reshape(S, D)
            acc = part if acc is None else acc + part
        full[b] = acc
    return full


def _get_module():
    global _BUILT
    if _BUILT is None:
        _BUILT = build_module()
    return _BUILT


def _install_trace_shim():
    """This image's antenv lacks axon_hooks; provide the NTFF profile hook
    via ctypes so trace=True (or BASS_TRACE=1) works instead of crashing,
    and skip the artifact bucket upload."""
    try:
        import antenv.axon_hooks  # noqa: F401
        return
    except ImportError:
        pass
    import types
    import ctypes
    import contextlib

    so_path = "/opt/axon/libaxon_pjrt.so"
    mod = types.ModuleType("antenv.axon_hooks")
    try:
        lib = ctypes.CDLL(so_path)
        lib.axon_start_nrt_profile.argtypes = [
            ctypes.POINTER(ctypes.c_int64), ctypes.c_size_t]
        lib.axon_start_nrt_profile.restype = ctypes.c_int64
        lib.axon_stop_nrt_profile.argtypes = [ctypes.c_char_p]
        lib.axon_stop_nrt_profile.restype = ctypes.c_int64

        @contextlib.contextmanager
        def _hook(output_dir, device_ids):
            import jax
            jax.devices()
            if device_ids:
                ids = (ctypes.c_int64 * len(device_ids))(*device_ids)
                rc = lib.axon_start_nrt_profile(ids, len(device_ids))
            else:
                rc = lib.axon_start_nrt_profile(None, 0)
            if rc != 0:
                raise RuntimeError(f"axon_start_nrt_profile rc={rc}")
            try:
                yield
            finally:
                lib.axon_stop_nrt_profile(str(output_dir).encode())

        mod.get_axon_ntff_profile_hook = lambda: _hook
    except OSError:
        mod.get_axon_ntff_profile_hook = lambda: None
    mod.set_axon_ntff_profile_hook = lambda h: None
    sys.modules["antenv.axon_hooks"] = mod

    from concourse import bass_utils
    bass_utils.upload_artifacts = lambda tmpdir: tmpdir


def run_on_hw(in_maps, trace=False, trace_cores=None):
    _install_trace_shim()
    from concourse import bass_utils
    nc = _get_module()
    return bass_utils.run_bass_kernel_spmd(
        nc, in_maps, core_ids=list(range(N_CORES)),
        trace=trace, trace_cores=trace_cores)


def kernel(x, wq, wk, wv, wo):
    x = np.asarray(x, dtype=np.float32)
    wq = np.asarray(wq, dtype=np.float32)
    wk = np.asarray(wk, dtype=np.float32)
    wv = np.asarray(wv, dtype=np.float32)
    wo = np.asarray(wo, dtype=np.float32)
    in_maps = make_in_maps(x, wq, wk, wv, wo)
    res = run_on_hw(in_maps, trace=False)
    return assemble_output(res.results)



# revision 32
# speedup vs baseline: 1.0018x; 1.0018x over previous
"""Llama attention layer (B=2, S=2048, D=2048, H=16, HD=128, RoPE, causal)
on 8 Trainium2 NeuronCores.

Sharding: core c -> (batch b = c//4, head group g = c%4 of 4 heads).
Each core computes q/k/v projections for its 512 columns of wq/wk/wv,
RoPE, causal attention for its 4 heads, and the out-projection against
its 512 rows of wo (a partial sum over head groups). The host sums the
4 partials per batch and stacks the 2 batches.

All device matmuls run in bf16 with fp32 PSUM accumulation. Softmax is
computed without max-subtraction (scores here are bounded ~|9|), with
the denominator obtained from an M=1 ones-matmul over exp(scores^T).

Perf structure (vs the naive version):
- xT is packed m-major on the host so each 128-column block of x^T is
  one contiguous DMA; the V-projection m-groups start as soon as their
  own block lands instead of waiting for all of xT.
- Diagonal 128x512 score/exp/attn-V work is narrowed to the causally
  valid query columns; the V-matmul is split per 128-col region so each
  region's accumulation group can close with its own stop flag.
- Softmax denominator: DVE adds exp-chunk pairs, one ones-matmul per
  pair accumulates into PSUM. (Deeper reduction trees on GpSimd were
  tried and regress: the PE executes in program order, so a ones-matmul
  that waits on a ~1.2us GpSimd add stalls every later matmul.)
- Output partials are written bf16 (host accumulates in fp32), halving
  output DMA.
"""

import os
import sys

import numpy as np
import ml_dtypes

if "/opt/trn_rl_repo" not in sys.path:
    sys.path.insert(0, "/opt/trn_rl_repo")

import concourse.bass as bass  # noqa: E402
import concourse.mybir as mybir  # noqa: E402
import concourse.bacc as bacc  # noqa: E402
import concourse.tile as tile  # noqa: E402

BF16 = ml_dtypes.bfloat16

B, S, D, H = 2, 2048, 2048, 16
HD = D // H            # 128, head dim
G = 4                  # head groups (cores per batch)
NH = H // G            # 4 heads per core
DG = NH * HD           # 512, per-core head width
P = 128
KO = D // P            # 16 k-subtiles over D
NKT = S // P           # 16 key chunks of 128
NQT = S // 512         # 4 q tiles of 512
QT = 512
ROPE_THETA = 10000.0
SCALE = 1.0 / float(np.sqrt(HD))

N_CORES = 8

_BUILT = None  # (nc,) cache


def build_module():
    fp32 = mybir.dt.float32
    bf16 = mybir.dt.bfloat16

    nc = bacc.Bacc("TRN2", target_bir_lowering=False, debug=False,
                   num_devices=N_CORES, num_swdge_queues=4)

    xT = nc.dram_tensor("xT", [P, NKT, KO, P], bf16, kind="ExternalInput")
    wq = nc.dram_tensor("wq", [P, KO, DG], bf16, kind="ExternalInput")
    wk = nc.dram_tensor("wk", [P, KO, DG], bf16, kind="ExternalInput")
    wv = nc.dram_tensor("wv", [P, KO, DG], bf16, kind="ExternalInput")
    wo = nc.dram_tensor("wo", [P, NH, D], bf16, kind="ExternalInput")
    cosT = nc.dram_tensor("cosT", [P, S], bf16, kind="ExternalInput")
    sinT = nc.dram_tensor("sinT", [P, S], bf16, kind="ExternalInput")
    maskT = nc.dram_tensor("maskT", [P, P], bf16, kind="ExternalInput")
    out = nc.dram_tensor("out", [P, NKT, D], bf16, kind="ExternalOutput")

    Exp = mybir.ActivationFunctionType.Exp

    with tile.TileContext(nc) as tc:
        with tc.tile_pool(name="const", bufs=1) as const, \
             tc.tile_pool(name="big", bufs=1) as big:
            ones = const.tile([P, P], bf16)
            nc.vector.memset(ones, 1.0)
            # dummy exp so the ACT Exp table loads during the DMA prefix,
            # not at the first real exp in the attention phase
            warm = const.tile([1, 1], fp32)
            nc.scalar.activation(warm, ones[0:1, 0:1],
                                 mybir.ActivationFunctionType.Exp)
            warm_mm = const.tile([P, QT], bf16)
            nc.vector.memset(warm_mm, 0.0)

            qT_sb = big.tile([P, NH, S], bf16)   # per head: [HD, S]
            kT_sb = big.tile([P, NH, S], bf16)
            v_sb = big.tile([P, NKT, DG], bf16)  # [key%128, keychunk, dg]
            wo_sb = big.tile([P, NH, D], bf16)
            mask_sb = const.tile([P, P], bf16)
            ao0_sb = big.tile([P, 2, QT], bf16)  # (qt=0, h=0/1) attention out

            def attend(qt, h, pool_s, s_bufs, pool_o, pool_sum, pool_ax,
                       ax_bufs, pool_axp, axp_bufs, pool_axq, pool_ep,
                       ep_bufs, dst):
                """Causal attention for one (q-tile, head) into dst."""
                n_kt = 4 * (qt + 1)  # causal: key chunks 0..n_kt-1
                ps_o = pool_o.tile([P, QT], fp32, tag="ps_o")
                # all-ones [128,128] lhsT -> every psum row holds sumexp:
                # no partition-broadcast needed later
                ps_sum = pool_sum.tile([P, QT], fp32, tag="ps_sum")
                # denominator plan: early chunk pairs are added into quads
                # on DVE and their ones-matmul is DEFERRED 4 chunks (so the
                # PE, which executes in program order, never waits on the
                # DVE adds); the last 4 chunks use immediate pair matmuls.
                n_equads = max(0, n_kt - 4) // 4
                mm_total = n_equads + 2
                mm_i = 0
                pending = {}
                ax_prev = None
                pair_prev = None
                for m in range(n_kt):
                    if m in pending:
                        nc.tensor.matmul(ps_sum, ones, pending.pop(m),
                                         start=(mm_i == 0),
                                         stop=(mm_i == mm_total - 1))
                        mm_i += 1
                    # diagonal chunks only cover queries >= their first key:
                    # narrow to columns [colo:QT)
                    o = m - qt * 4
                    colo = max(0, o) * P
                    ps_s = pool_s.tile([P, QT], fp32, tag="ps_s",
                                       bufs=s_bufs)
                    nc.tensor.matmul(ps_s[:, colo:],
                                     kT_sb[:, h, m * P:(m + 1) * P],
                                     qT_sb[:, h,
                                           qt * QT + colo:(qt + 1) * QT],
                                     start=True, stop=True)
                    ax = pool_ax.tile([P, QT], bf16, tag="ax", bufs=ax_bufs)
                    if colo:
                        # zero the causally-dead prefix so the denominator
                        # adds see zeros there
                        nc.gpsimd.memset(ax[:, 0:colo], 0.0)
                    nc.scalar.activation(ax[:, colo:], ps_s[:, colo:],
                                         Exp, scale=SCALE)
                    if o >= 0:
                        # triangular mask on the 128 cols that straddle the
                        # diagonal; later cols are fully valid
                        nc.vector.tensor_mul(ax[:, colo:colo + P],
                                             ax[:, colo:colo + P], mask_sb)
                    # attn @ V, narrowed. Each 128-col region's last
                    # contribution is the diagonal chunk o = region index,
                    # so that slice carries stop=True while the rest keeps
                    # accumulating.
                    vsl = v_sb[:, m, h * HD:(h + 1) * HD]
                    if o < 0:
                        nc.tensor.matmul(ps_o, vsl, ax,
                                         start=(m == 0), stop=False)
                    else:
                        nc.tensor.matmul(ps_o[:, colo:colo + P], vsl,
                                         ax[:, colo:colo + P],
                                         start=(m == 0), stop=True)
                        if colo + P < QT:
                            nc.tensor.matmul(ps_o[:, colo + P:], vsl,
                                             ax[:, colo + P:],
                                             start=(m == 0), stop=False)
                    if m % 2 == 0:
                        ax_prev = ax
                    else:
                        pair = pool_axp.tile([P, QT], bf16, tag="axp",
                                             bufs=axp_bufs)
                        nc.vector.tensor_add(pair, ax_prev, ax)
                        if m >= n_kt - 4:
                            nc.tensor.matmul(ps_sum, ones, pair,
                                             start=(mm_i == 0),
                                             stop=(mm_i == mm_total - 1))
                            mm_i += 1
                        elif m % 4 == 1:
                            pair_prev = pair
                        else:
                            quad = pool_axq.tile([P, QT], bf16, tag="axq",
                                                 bufs=3)
                            nc.vector.tensor_add(quad, pair_prev, pair)
                            pending[min(m + 4, n_kt - 1)] = quad
                rec = pool_ep.tile([P, QT], fp32, tag="rec", bufs=ep_bufs)
                nc.vector.reciprocal_approx_fast(rec, ps_sum)
                nc.vector.tensor_mul(dst, ps_o, rec)

            # ---------------- phase 1: projections + RoPE ----------------
            with tc.tile_pool(name="w_pool", bufs=1) as w_pool, \
                 tc.tile_pool(name="rope", bufs=4) as rope, \
                 tc.tile_pool(name="ps1", bufs=1, space="PSUM") as ps1:
                # DMA order matters: wv first (V-loop gate), then xT in
                # m-major column blocks (the dram layout is packed so block m
                # is contiguous) so V m-group m only waits for its own block,
                # then the K/Q-phase tensors, then phase-2/3 tensors.
                wv_sb = w_pool.tile([P, KO, DG], bf16)
                # m-major like the DRAM packing: per-block DMA is contiguous
                # (4KB/partition). A k-major SBUF layout would make the block
                # DMA scatter 256B lines, which runs ~7x slower.
                xT_sb = w_pool.tile([P, NKT, KO, P], bf16)
                wk_sb = w_pool.tile([P, KO, DG], bf16)
                cos_sb = w_pool.tile([P, S], bf16)
                sin_sb = w_pool.tile([P, S], bf16)
                wq_sb = w_pool.tile([P, KO, DG], bf16)
                # Descriptor generation costs ~0.6us of sequencer time per
                # dma_start, so split the input stream over BOTH hardware DGE
                # queues: the weight stream on the Activation queue (idle in
                # phase 1), the xT column blocks on the SP queue. The V-loop
                # m-group m then only waits for wv (k-sliced, so its first
                # matmuls start early) and its own xT block.
                for ks_ in range(0, KO, 4):
                    nc.scalar.dma_start(wv_sb[:, ks_:ks_ + 4, :],
                                        wv.ap()[:, ks_:ks_ + 4, :])
                for m in range(NKT):
                    nc.sync.dma_start(xT_sb[:, m], xT.ap()[:, m])
                nc.scalar.dma_start(wk_sb, wk.ap())
                nc.scalar.dma_start(cos_sb, cosT.ap())
                nc.scalar.dma_start(sin_sb, sinT.ap())
                nc.scalar.dma_start(wq_sb, wq.ap())
                nc.sync.dma_start(mask_sb, maskT.ap())
                nc.sync.dma_start(wo_sb, wo.ap())

                # PE p-state warmup: the tensor engine clocks up only after
                # ~3us of continuous execution, so chew on zeros while the
                # first wv/xT DMAs land -- the first real matmuls then run at
                # full clock instead of half.
                ps_w = ps1.tile([P, QT], fp32, tag="psv", bufs=4)
                for r in range(5):
                    nc.tensor.matmul(ps_w, ones, warm_mm,
                                     start=(r == 0), stop=(r == 4))

                # V: [keys, dg] natural layout, keychunk tiles of 128.
                # k-OUTER waves of 4 m-groups (4 psum banks): each wv k-slice
                # unlocks 4 matmuls, so the PE ramps as the k-sliced wv DMA
                # trickles in instead of waiting for all of wv.
                for mw in range(0, NKT, 4):
                    pss = [ps1.tile([P, DG], fp32, tag="psv",
                                    name=f"psv{i}", bufs=4) for i in range(4)]
                    for k in range(KO):
                        for i in range(4):
                            nc.tensor.matmul(pss[i], xT_sb[:, mw + i, k, :],
                                             wv_sb[:, k, :],
                                             start=(k == 0),
                                             stop=(k == KO - 1))
                    for i in range(4):
                        nc.vector.tensor_copy(v_sb[:, mw + i, :], pss[i])

                # K then Q: [HD, S] transposed layout + RoPE.
                # Heads processed in pairs so the two psum tags can be
                # double-buffered (2 tags x 2 bufs) -- RoPE of one pair
                # overlaps the matmuls of the next.
                for which, w_sb, dstT in (("k", wk_sb, kT_sb), ("q", wq_sb, qT_sb)):
                    for nt2 in range(2 * NQT):
                        nt, hp = divmod(nt2, 2)
                        sl = slice(nt * QT, (nt + 1) * QT)
                        heads = (2 * hp, 2 * hp + 1)
                        # share the "psv" tag (banks 0-3) so all of phase 1
                        # stays within 4 psum banks, leaving 4-7 free for
                        # the attention pools to start without bank conflicts
                        pss = {}
                        for h in heads:
                            pss[h] = ps1.tile([P, QT], fp32, tag="psv",
                                              name=f"psp{h}", bufs=4)
                        # rhs: the nt-th 512 queries = xT m-blocks 4nt..4nt+3
                        # at fixed k -- a strided [128, 4, 128] AP
                        for k in range(KO):
                            for h in heads:
                                nc.tensor.matmul(
                                    pss[h], w_sb[:, k, h * HD:(h + 1) * HD],
                                    xT_sb[:, nt * 4:(nt + 1) * 4, k, :],
                                    start=(k == 0), stop=(k == KO - 1))
                        for h in heads:
                            ps = pss[h]
                            dst = dstT[:, h, sl]
                            # rope: dst = ps * cos + swap(ps) * sin_signed.
                            # The swapped reads must come from PSUM (the SB-SB
                            # same-base-partition rule forbids them on SBUF);
                            # the straight read goes via a parallel ACT copy so
                            # the psum bank drains fast.
                            tmp = rope.tile([P, QT], bf16, tag="tmp")
                            nc.vector.tensor_mul(tmp[0:64], ps[64:128],
                                                 sin_sb[0:64, sl])
                            nc.vector.tensor_mul(tmp[64:128], ps[0:64],
                                                 sin_sb[64:128, sl])
                            qb = rope.tile([P, QT], bf16, tag="qb")
                            nc.scalar.copy(qb, ps)
                            nc.vector.tensor_mul(dst, qb, cos_sb[:, sl])
                            nc.vector.tensor_add(dst, dst, tmp)

                # attention for (qt=0, h=0/1) right here: its inputs (kT/qT
                # tile 0, v) are long ready, and PSUM banks 4-7 are free
                # (phase 1 keeps to 4 banks via the shared psv tag). The PE
                # chews on these chunks while phase-1 PSUM drains and the
                # phase-2 pools open, bridging the transition gap.
                with tc.tile_pool(name="a0ps", bufs=1, space="PSUM") as a0ps, \
                     tc.tile_pool(name="a0sb", bufs=1) as a0sb:
                    for h0 in range(2):
                        attend(0, h0, a0ps, 2, a0ps, a0ps, a0sb, 4, a0sb, 2,
                               a0sb, a0sb, 1, ao0_sb[:, h0, :])

            # ---------------- phases 2+3 ----------------
            with tc.tile_pool(name="big2", bufs=1) as big2:
                aoT_sb = big2.tile([P, NH, S], bf16)  # attention out^T

                # phases 2+3 interleaved: attention for q-tile qt, then the
                # out-projection rows it unblocks (their matmuls have no ACT
                # dependency and fill the exp-latency bubbles)
                with tc.tile_pool(name="ax_pool", bufs=20) as ax_pool, \
                     tc.tile_pool(name="axp_pool", bufs=6) as axp_pool, \
                     tc.tile_pool(name="axq_pool", bufs=3) as axq_pool, \
                     tc.tile_pool(name="ep", bufs=4) as ep, \
                     tc.tile_pool(name="stage", bufs=6) as stage, \
                     tc.tile_pool(name="ps2s", bufs=3, space="PSUM") as ps2s, \
                     tc.tile_pool(name="ps2o", bufs=2, space="PSUM") as ps2o, \
                     tc.tile_pool(name="ps2", bufs=1, space="PSUM") as ps2, \
                     tc.tile_pool(name="ps3", bufs=2, space="PSUM") as ps3:
                    def emit_outproj(qo):
                        for n in range(D // QT):
                            nsl = slice(n * QT, (n + 1) * QT)
                            ps = ps3.tile([P, QT], fp32, tag="ps_out")
                            for h in range(NH):
                                if h < 2 and qo < 4:
                                    lhs = ao0_sb[:, h, qo * P:(qo + 1) * P]
                                else:
                                    lhs = aoT_sb[:, h, qo * P:(qo + 1) * P]
                                nc.tensor.matmul(
                                    ps, lhs, wo_sb[:, h, nsl],
                                    start=(h == 0), stop=(h == NH - 1))
                            ob = stage.tile([P, QT], bf16, tag="ob")
                            nc.vector.tensor_copy(ob, ps)
                            nc.sync.dma_start(out.ap()[:, qo, nsl], ob)

                    for qt in range(NQT):
                        qsl = slice(qt * QT, (qt + 1) * QT)
                        for h in range(NH):
                            if qt == 0 and h < 2:
                                continue  # computed at the end of phase 1
                            attend(qt, h, ps2s, 3, ps2o, ps2, ax_pool, 20,
                                   axp_pool, 6, axq_pool, ep, 4,
                                   aoT_sb[:, h, qsl])

                            # out-projection for the previous q-tile's rows,
                            # interleaved between this tile's heads so the
                            # normalize latency of the previous tile's last
                            # head hides under this head's chunk matmuls
                            if qt > 0:
                                emit_outproj(4 * (qt - 1) + h)

                    # the last q-tile's rows have nothing to hide behind
                    for qo in range(4 * (NQT - 1), 4 * NQT):
                        emit_outproj(qo)

    nc.compile()
    return nc


def _rope_tables():
    inv_freq = 1.0 / (ROPE_THETA ** (np.arange(0, HD, 2, dtype=np.float64) / HD))
    pos = np.arange(S, dtype=np.float64)
    freqs = np.outer(pos, inv_freq)                    # [S, HD/2]
    emb = np.concatenate([freqs, freqs], axis=-1)      # [S, HD]
    cos = np.cos(emb).T.astype(BF16)                   # [HD, S]
    sin = np.sin(emb).T.astype(np.float32)
    sin[: HD // 2] *= -1.0                             # fold rotate_half sign
    return cos, sin.astype(BF16)


def _pack_kd(a):
    """[D, N] -> [P, D//P, N] with d = ko*P + p."""
    d, n = a.shape
    return np.ascontiguousarray(
        a.reshape(d // P, P, n).transpose(1, 0, 2)).astype(BF16)


def _pack_xT(xb):
    """x[b] [S, D] -> [P, NKT, KO, P] m-major so each 128-col block of x^T
    is one contiguous DMA."""
    t = _pack_kd(np.ascontiguousarray(xb.T))           # [P, KO, S]
    return np.ascontiguousarray(
        t.reshape(P, KO, NKT, P).transpose(0, 2, 1, 3))


def make_in_maps(x, wq, wk, wv, wo):
    cosT, sinT = _rope_tables()
    i = np.arange(P)[:, None]
    j = np.arange(P)[None, :]
    mask = (i <= j).astype(BF16)

    xT_packed = [_pack_xT(x[b]) for b in range(B)]
    in_maps = []
    for c in range(N_CORES):
        b, g = divmod(c, G)
        gsl = slice(g * DG, (g + 1) * DG)
        in_maps.append({
            "xT": xT_packed[b],
            "wq": _pack_kd(wq[:, gsl]),
            "wk": _pack_kd(wk[:, gsl]),
            "wv": _pack_kd(wv[:, gsl]),
            "wo": _pack_kd(np.ascontiguousarray(wo[gsl, :])),
            "cosT": cosT,
            "sinT": sinT,
            "maskT": mask,
        })
    return in_maps


def assemble_output(results):
    """results: list of 8 dicts with 'out' [P, NKT, D] bf16 partials."""
    full = np.empty((B, S, D), dtype=np.float32)
    for b in range(B):
        acc = None
        for g in range(G):
            r = results[b * G + g]["out"].astype(np.float32)
            part = r.transpose(1, 0, 2).reshape(S, D)
            acc = part if acc is None else acc + part
        full[b] = acc
    return full


def _get_module():
    global _BUILT
    if _BUILT is None:
        _BUILT = build_module()
    return _BUILT


def _install_trace_shim():
    """This image's antenv lacks axon_hooks; provide the NTFF profile hook
    via ctypes so trace=True (or BASS_TRACE=1) works instead of crashing,
    and skip the artifact bucket upload."""
    try:
        import antenv.axon_hooks  # noqa: F401
        return
    except ImportError:
        pass
    import types
    import ctypes
    import contextlib

    so_path = "/opt/axon/libaxon_pjrt.so"
    mod = types.ModuleType("antenv.axon_hooks")
    try:
        lib = ctypes.CDLL(so_path)
        lib.axon_start_nrt_profile.argtypes = [
            ctypes.POINTER(ctypes.c_int64), ctypes.c_size_t]
        lib.axon_start_nrt_profile.restype = ctypes.c_int64
        lib.axon_stop_nrt_profile.argtypes = [ctypes.c_char_p]
        lib.axon_stop_nrt_profile.restype = ctypes.c_int64

        @contextlib.contextmanager
        def _hook(output_dir, device_ids):
            import jax
            jax.devices()
            if device_ids:
                ids = (ctypes.c_int64 * len(device_ids))(*device_ids)
                rc = lib.axon_start_nrt_profile(ids, len(device_ids))
            else:
                rc = lib.axon_start_nrt_profile(None, 0)
            if rc != 0:
                raise RuntimeError(f"axon_start_nrt_profile rc={rc}")
            try:
                yield
            finally:
                lib.axon_stop_nrt_profile(str(output_dir).encode())

        mod.get_axon_ntff_profile_hook = lambda: _hook
    except OSError:
        mod.get_axon_ntff_profile_hook = lambda: None
    mod.set_axon_ntff_profile_hook = lambda h: None
    sys.modules["antenv.axon_hooks"] = mod

    from concourse import bass_utils
    bass_utils.upload_artifacts = lambda tmpdir: tmpdir


def run_on_hw(in_maps, trace=False, trace_cores=None):
    _install_trace_shim()
    from concourse import bass_utils
    nc = _get_module()
    return bass_utils.run_bass_kernel_spmd(
        nc, in_maps, core_ids=list(range(N_CORES)),
        trace=trace, trace_cores=trace_cores)


def kernel(x, wq, wk, wv, wo):
    x = np.asarray(x, dtype=np.float32)
    wq = np.asarray(wq, dtype=np.float32)
    wk = np.asarray(wk, dtype=np.float32)
    wv = np.asarray(wv, dtype=np.float32)
    wo = np.asarray(wo, dtype=np.float32)
    in_maps = make_in_maps(x, wq, wk, wv, wo)
    res = run_on_hw(in_maps, trace=False)
    return assemble_output(res.results)



# revision 33
# speedup vs baseline: 1.0055x; 1.0036x over previous
"""Llama attention layer (B=2, S=2048, D=2048, H=16, HD=128, RoPE, causal)
on 8 Trainium2 NeuronCores.

Sharding: core c -> (batch b = c//4, head group g = c%4 of 4 heads).
Each core computes q/k/v projections for its 512 columns of wq/wk/wv,
RoPE, causal attention for its 4 heads, and the out-projection against
its 512 rows of wo (a partial sum over head groups). The host sums the
4 partials per batch and stacks the 2 batches.

All device matmuls run in bf16 with fp32 PSUM accumulation. Softmax is
computed without max-subtraction (scores here are bounded ~|9|), with
the denominator obtained from an M=1 ones-matmul over exp(scores^T).

Perf structure (vs the naive version; ~353us -> ~326us):
- xT is packed m-major on the host AND in SBUF so every DMA line is
  contiguous (strided 256B-line SBUF writes run ~7x slower); the
  K/Q-projection matmuls read it through a strided [128,4,128] rhs AP.
- Input DMAs are split over both hardware DGE queues (SP + Activation)
  because descriptor generation costs ~0.6us of sequencer time each.
- The V-projection runs k-outer over waves of 4 PSUM banks so each wv
  k-slice unlocks matmuls while the DMA trickles in; a few warmup
  matmuls on zeros pin the PE p-state early (the PE only reaches full
  clock after ~3us of continuous execution).
- Diagonal 128x512 score/exp/attn-V work is narrowed to the causally
  valid query columns; the V-matmul is split per 128-col region so each
  region's accumulation group can close with its own stop flag.
- Softmax denominator: DVE adds exp-chunk pairs, then pairs into quads;
  each quad's ones-matmul is deferred 4 chunks so the in-program-order
  PE never waits on the DVE adds. The last 4 chunks use immediate pair
  matmuls so the reciprocal isn't delayed.
- Attention for (qt=0, heads 0-1) runs at the tail of phase 1 on the 4
  free PSUM banks, hiding the phase-1 PSUM drain / phase-2 pool-open
  latency; each q-tile's out-projection is interleaved between the next
  tile's heads to hide the normalize latency.
- Output partials are written bf16 (host accumulates in fp32), halving
  output DMA.
"""

import os
import sys

import numpy as np
import ml_dtypes

if "/opt/trn_rl_repo" not in sys.path:
    sys.path.insert(0, "/opt/trn_rl_repo")

import concourse.bass as bass  # noqa: E402
import concourse.mybir as mybir  # noqa: E402
import concourse.bacc as bacc  # noqa: E402
import concourse.tile as tile  # noqa: E402

BF16 = ml_dtypes.bfloat16

B, S, D, H = 2, 2048, 2048, 16
HD = D // H            # 128, head dim
G = 4                  # head groups (cores per batch)
NH = H // G            # 4 heads per core
DG = NH * HD           # 512, per-core head width
P = 128
KO = D // P            # 16 k-subtiles over D
NKT = S // P           # 16 key chunks of 128
NQT = S // 512         # 4 q tiles of 512
QT = 512
ROPE_THETA = 10000.0
SCALE = 1.0 / float(np.sqrt(HD))

N_CORES = 8

_BUILT = None  # (nc,) cache


def build_module():
    fp32 = mybir.dt.float32
    bf16 = mybir.dt.bfloat16

    nc = bacc.Bacc("TRN2", target_bir_lowering=False, debug=False,
                   num_devices=N_CORES, num_swdge_queues=4)

    xT = nc.dram_tensor("xT", [P, NKT, KO, P], bf16, kind="ExternalInput")
    wq = nc.dram_tensor("wq", [P, KO, DG], bf16, kind="ExternalInput")
    wk = nc.dram_tensor("wk", [P, KO, DG], bf16, kind="ExternalInput")
    wv = nc.dram_tensor("wv", [P, KO, DG], bf16, kind="ExternalInput")
    wo = nc.dram_tensor("wo", [P, NH, D], bf16, kind="ExternalInput")
    cosT = nc.dram_tensor("cosT", [P, S], bf16, kind="ExternalInput")
    sinT = nc.dram_tensor("sinT", [P, S], bf16, kind="ExternalInput")
    maskT = nc.dram_tensor("maskT", [P, P], bf16, kind="ExternalInput")
    out = nc.dram_tensor("out", [P, NKT, D], bf16, kind="ExternalOutput")

    Exp = mybir.ActivationFunctionType.Exp

    with tile.TileContext(nc) as tc:
        with tc.tile_pool(name="const", bufs=1) as const, \
             tc.tile_pool(name="big", bufs=1) as big:
            ones = const.tile([P, P], bf16)
            nc.vector.memset(ones, 1.0)
            # dummy exp so the ACT Exp table loads during the DMA prefix,
            # not at the first real exp in the attention phase
            warm = const.tile([1, 1], fp32)
            nc.scalar.activation(warm, ones[0:1, 0:1],
                                 mybir.ActivationFunctionType.Exp)
            warm_mm = const.tile([P, QT], bf16)
            nc.vector.memset(warm_mm, 0.0)

            qT_sb = big.tile([P, NH, S], bf16)   # per head: [HD, S]
            kT_sb = big.tile([P, NH, S], bf16)
            v_sb = big.tile([P, NKT, DG], bf16)  # [key%128, keychunk, dg]
            wo_sb = big.tile([P, NH, D], bf16)
            mask_sb = const.tile([P, P], bf16)
            ao0_sb = big.tile([P, 2, QT], bf16)  # (qt=0, h=0/1) attention out

            def attend(qt, h, pool_s, s_bufs, pool_o, pool_sum, pool_ax,
                       ax_bufs, pool_axp, axp_bufs, pool_axq, pool_ep,
                       ep_bufs, dst):
                """Causal attention for one (q-tile, head) into dst."""
                n_kt = 4 * (qt + 1)  # causal: key chunks 0..n_kt-1
                ps_o = pool_o.tile([P, QT], fp32, tag="ps_o")
                # all-ones [128,128] lhsT -> every psum row holds sumexp:
                # no partition-broadcast needed later
                ps_sum = pool_sum.tile([P, QT], fp32, tag="ps_sum")
                # denominator plan: early chunk pairs are added into quads
                # on DVE and their ones-matmul is DEFERRED 4 chunks (so the
                # PE, which executes in program order, never waits on the
                # DVE adds); the last 4 chunks use immediate pair matmuls.
                n_equads = max(0, n_kt - 4) // 4
                mm_total = n_equads + 2
                mm_i = 0
                pending = {}
                ax_prev = None
                pair_prev = None
                for m in range(n_kt):
                    if m in pending:
                        nc.tensor.matmul(ps_sum, ones, pending.pop(m),
                                         start=(mm_i == 0),
                                         stop=(mm_i == mm_total - 1))
                        mm_i += 1
                    # diagonal chunks only cover queries >= their first key:
                    # narrow to columns [colo:QT)
                    o = m - qt * 4
                    colo = max(0, o) * P
                    ps_s = pool_s.tile([P, QT], fp32, tag="ps_s",
                                       bufs=s_bufs)
                    nc.tensor.matmul(ps_s[:, colo:],
                                     kT_sb[:, h, m * P:(m + 1) * P],
                                     qT_sb[:, h,
                                           qt * QT + colo:(qt + 1) * QT],
                                     start=True, stop=True)
                    ax = pool_ax.tile([P, QT], bf16, tag="ax", bufs=ax_bufs)
                    if colo:
                        # zero the causally-dead prefix so the denominator
                        # adds see zeros there
                        nc.gpsimd.memset(ax[:, 0:colo], 0.0)
                    nc.scalar.activation(ax[:, colo:], ps_s[:, colo:],
                                         Exp, scale=SCALE)
                    if o >= 0:
                        # triangular mask on the 128 cols that straddle the
                        # diagonal; later cols are fully valid
                        nc.vector.tensor_mul(ax[:, colo:colo + P],
                                             ax[:, colo:colo + P], mask_sb)
                    # attn @ V, narrowed. Each 128-col region's last
                    # contribution is the diagonal chunk o = region index,
                    # so that slice carries stop=True while the rest keeps
                    # accumulating.
                    vsl = v_sb[:, m, h * HD:(h + 1) * HD]
                    if o < 0:
                        nc.tensor.matmul(ps_o, vsl, ax,
                                         start=(m == 0), stop=False)
                    else:
                        nc.tensor.matmul(ps_o[:, colo:colo + P], vsl,
                                         ax[:, colo:colo + P],
                                         start=(m == 0), stop=True)
                        if colo + P < QT:
                            nc.tensor.matmul(ps_o[:, colo + P:], vsl,
                                             ax[:, colo + P:],
                                             start=(m == 0), stop=False)
                    if m % 2 == 0:
                        ax_prev = ax
                    else:
                        pair = pool_axp.tile([P, QT], bf16, tag="axp",
                                             bufs=axp_bufs)
                        nc.vector.tensor_add(pair, ax_prev, ax)
                        if m >= n_kt - 4:
                            nc.tensor.matmul(ps_sum, ones, pair,
                                             start=(mm_i == 0),
                                             stop=(mm_i == mm_total - 1))
                            mm_i += 1
                        elif m % 4 == 1:
                            pair_prev = pair
                        else:
                            quad = pool_axq.tile([P, QT], bf16, tag="axq",
                                                 bufs=3)
                            nc.vector.tensor_add(quad, pair_prev, pair)
                            pending[min(m + 4, n_kt - 1)] = quad
                rec = pool_ep.tile([P, QT], fp32, tag="rec", bufs=ep_bufs)
                nc.vector.reciprocal_approx_fast(rec, ps_sum)
                nc.vector.tensor_mul(dst, ps_o, rec)

            # ---------------- phase 1: projections + RoPE ----------------
            with tc.tile_pool(name="w_pool", bufs=1) as w_pool, \
                 tc.tile_pool(name="rope", bufs=4) as rope, \
                 tc.tile_pool(name="ps1", bufs=1, space="PSUM") as ps1:
                # DMA order matters: wv first (V-loop gate), then xT in
                # m-major column blocks (the dram layout is packed so block m
                # is contiguous) so V m-group m only waits for its own block,
                # then the K/Q-phase tensors, then phase-2/3 tensors.
                wv_sb = w_pool.tile([P, KO, DG], bf16)
                # m-major like the DRAM packing: per-block DMA is contiguous
                # (4KB/partition). A k-major SBUF layout would make the block
                # DMA scatter 256B lines, which runs ~7x slower.
                xT_sb = w_pool.tile([P, NKT, KO, P], bf16)
                wk_sb = w_pool.tile([P, KO, DG], bf16)
                cos_sb = w_pool.tile([P, S], bf16)
                sin_sb = w_pool.tile([P, S], bf16)
                wq_sb = w_pool.tile([P, KO, DG], bf16)
                # Descriptor generation costs ~0.6us of sequencer time per
                # dma_start, so split the input stream over BOTH hardware DGE
                # queues: the weight stream on the Activation queue (idle in
                # phase 1), the xT column blocks on the SP queue. The V-loop
                # m-group m then only waits for wv (k-sliced, so its first
                # matmuls start early) and its own xT block.
                for ks_ in range(0, KO, 4):
                    nc.scalar.dma_start(wv_sb[:, ks_:ks_ + 4, :],
                                        wv.ap()[:, ks_:ks_ + 4, :])
                for m in range(NKT):
                    nc.sync.dma_start(xT_sb[:, m], xT.ap()[:, m])
                nc.scalar.dma_start(wk_sb, wk.ap())
                nc.scalar.dma_start(cos_sb, cosT.ap())
                nc.scalar.dma_start(sin_sb, sinT.ap())
                nc.scalar.dma_start(wq_sb, wq.ap())
                nc.sync.dma_start(mask_sb, maskT.ap())
                nc.sync.dma_start(wo_sb, wo.ap())

                # PE p-state warmup: the tensor engine clocks up only after
                # ~3us of continuous execution, so chew on zeros while the
                # first wv/xT DMAs land -- the first real matmuls then run at
                # full clock instead of half.
                ps_w = ps1.tile([P, QT], fp32, tag="psv", bufs=4)
                for r in range(5):
                    nc.tensor.matmul(ps_w, ones, warm_mm,
                                     start=(r == 0), stop=(r == 4))

                # V: [keys, dg] natural layout, keychunk tiles of 128.
                # k-OUTER waves of 4 m-groups (4 psum banks): each wv k-slice
                # unlocks 4 matmuls, so the PE ramps as the k-sliced wv DMA
                # trickles in instead of waiting for all of wv.
                for mw in range(0, NKT, 4):
                    pss = [ps1.tile([P, DG], fp32, tag="psv",
                                    name=f"psv{i}", bufs=4) for i in range(4)]
                    for k in range(KO):
                        for i in range(4):
                            nc.tensor.matmul(pss[i], xT_sb[:, mw + i, k, :],
                                             wv_sb[:, k, :],
                                             start=(k == 0),
                                             stop=(k == KO - 1))
                    for i in range(4):
                        nc.vector.tensor_copy(v_sb[:, mw + i, :], pss[i])

                # K then Q: [HD, S] transposed layout + RoPE.
                # Heads processed in pairs so the two psum tags can be
                # double-buffered (2 tags x 2 bufs) -- RoPE of one pair
                # overlaps the matmuls of the next.
                for which, w_sb, dstT in (("k", wk_sb, kT_sb), ("q", wq_sb, qT_sb)):
                    for nt2 in range(2 * NQT):
                        nt, hp = divmod(nt2, 2)
                        sl = slice(nt * QT, (nt + 1) * QT)
                        heads = (2 * hp, 2 * hp + 1)
                        # share the "psv" tag (banks 0-3) so all of phase 1
                        # stays within 4 psum banks, leaving 4-7 free for
                        # the attention pools to start without bank conflicts
                        pss = {}
                        for h in heads:
                            pss[h] = ps1.tile([P, QT], fp32, tag="psv",
                                              name=f"psp{h}", bufs=4)
                        # rhs: the nt-th 512 queries = xT m-blocks 4nt..4nt+3
                        # at fixed k -- a strided [128, 4, 128] AP
                        for k in range(KO):
                            for h in heads:
                                nc.tensor.matmul(
                                    pss[h], w_sb[:, k, h * HD:(h + 1) * HD],
                                    xT_sb[:, nt * 4:(nt + 1) * 4, k, :],
                                    start=(k == 0), stop=(k == KO - 1))
                        for h in heads:
                            ps = pss[h]
                            dst = dstT[:, h, sl]
                            # rope: dst = ps * cos + swap(ps) * sin_signed.
                            # The swapped reads must come from PSUM (the SB-SB
                            # same-base-partition rule forbids them on SBUF);
                            # the straight read goes via a parallel ACT copy so
                            # the psum bank drains fast.
                            tmp = rope.tile([P, QT], bf16, tag="tmp")
                            nc.vector.tensor_mul(tmp[0:64], ps[64:128],
                                                 sin_sb[0:64, sl])
                            nc.vector.tensor_mul(tmp[64:128], ps[0:64],
                                                 sin_sb[64:128, sl])
                            qb = rope.tile([P, QT], bf16, tag="qb")
                            nc.scalar.copy(qb, ps)
                            nc.vector.tensor_mul(dst, qb, cos_sb[:, sl])
                            nc.vector.tensor_add(dst, dst, tmp)

                # attention for (qt=0, h=0/1) right here: its inputs (kT/qT
                # tile 0, v) are long ready, and PSUM banks 4-7 are free
                # (phase 1 keeps to 4 banks via the shared psv tag). The PE
                # chews on these chunks while phase-1 PSUM drains and the
                # phase-2 pools open, bridging the transition gap.
                with tc.tile_pool(name="a0ps", bufs=1, space="PSUM") as a0ps, \
                     tc.tile_pool(name="a0sb", bufs=1) as a0sb:
                    for h0 in range(2):
                        attend(0, h0, a0ps, 2, a0ps, a0ps, a0sb, 4, a0sb, 2,
                               a0sb, a0sb, 1, ao0_sb[:, h0, :])

            # ---------------- phases 2+3 ----------------
            with tc.tile_pool(name="big2", bufs=1) as big2:
                aoT_sb = big2.tile([P, NH, S], bf16)  # attention out^T

                # phases 2+3 interleaved: attention for q-tile qt, then the
                # out-projection rows it unblocks (their matmuls have no ACT
                # dependency and fill the exp-latency bubbles)
                with tc.tile_pool(name="ax_pool", bufs=20) as ax_pool, \
                     tc.tile_pool(name="axp_pool", bufs=6) as axp_pool, \
                     tc.tile_pool(name="axq_pool", bufs=3) as axq_pool, \
                     tc.tile_pool(name="ep", bufs=4) as ep, \
                     tc.tile_pool(name="stage", bufs=6) as stage, \
                     tc.tile_pool(name="ps2s", bufs=3, space="PSUM") as ps2s, \
                     tc.tile_pool(name="ps2o", bufs=2, space="PSUM") as ps2o, \
                     tc.tile_pool(name="ps2", bufs=1, space="PSUM") as ps2, \
                     tc.tile_pool(name="ps3", bufs=2, space="PSUM") as ps3:
                    def emit_outproj(qo):
                        for n in range(D // QT):
                            nsl = slice(n * QT, (n + 1) * QT)
                            ps = ps3.tile([P, QT], fp32, tag="ps_out")
                            for h in range(NH):
                                if h < 2 and qo < 4:
                                    lhs = ao0_sb[:, h, qo * P:(qo + 1) * P]
                                else:
                                    lhs = aoT_sb[:, h, qo * P:(qo + 1) * P]
                                nc.tensor.matmul(
                                    ps, lhs, wo_sb[:, h, nsl],
                                    start=(h == 0), stop=(h == NH - 1))
                            ob = stage.tile([P, QT], bf16, tag="ob")
                            nc.vector.tensor_copy(ob, ps)
                            nc.sync.dma_start(out.ap()[:, qo, nsl], ob)

                    for qt in range(NQT):
                        qsl = slice(qt * QT, (qt + 1) * QT)
                        for h in range(NH):
                            if qt == 0 and h < 2:
                                continue  # computed at the end of phase 1
                            attend(qt, h, ps2s, 3, ps2o, ps2, ax_pool, 20,
                                   axp_pool, 6, axq_pool, ep, 4,
                                   aoT_sb[:, h, qsl])

                            # out-projection for the previous q-tile's rows,
                            # interleaved between this tile's heads so the
                            # normalize latency of the previous tile's last
                            # head hides under this head's chunk matmuls
                            if qt > 0:
                                emit_outproj(4 * (qt - 1) + h)

                    # the last q-tile's rows have nothing to hide behind
                    for qo in range(4 * (NQT - 1), 4 * NQT):
                        emit_outproj(qo)

    nc.compile()
    return nc


def _rope_tables():
    inv_freq = 1.0 / (ROPE_THETA ** (np.arange(0, HD, 2, dtype=np.float64) / HD))
    pos = np.arange(S, dtype=np.float64)
    freqs = np.outer(pos, inv_freq)                    # [S, HD/2]
    emb = np.concatenate([freqs, freqs], axis=-1)      # [S, HD]
    cos = np.cos(emb).T.astype(BF16)                   # [HD, S]
    sin = np.sin(emb).T.astype(np.float32)
    sin[: HD // 2] *= -1.0                             # fold rotate_half sign
    return cos, sin.astype(BF16)


def _pack_kd(a):
    """[D, N] -> [P, D//P, N] with d = ko*P + p."""
    d, n = a.shape
    return np.ascontiguousarray(
        a.reshape(d // P, P, n).transpose(1, 0, 2)).astype(BF16)


def _pack_xT(xb):
    """x[b] [S, D] -> [P, NKT, KO, P] m-major so each 128-col block of x^T
    is one contiguous DMA."""
    t = _pack_kd(np.ascontiguousarray(xb.T))           # [P, KO, S]
    return np.ascontiguousarray(
        t.reshape(P, KO, NKT, P).transpose(0, 2, 1, 3))


def make_in_maps(x, wq, wk, wv, wo):
    cosT, sinT = _rope_tables()
    i = np.arange(P)[:, None]
    j = np.arange(P)[None, :]
    mask = (i <= j).astype(BF16)

    xT_packed = [_pack_xT(x[b]) for b in range(B)]
    in_maps = []
    for c in range(N_CORES):
        b, g = divmod(c, G)
        gsl = slice(g * DG, (g + 1) * DG)
        in_maps.append({
            "xT": xT_packed[b],
            "wq": _pack_kd(wq[:, gsl]),
            "wk": _pack_kd(wk[:, gsl]),
            "wv": _pack_kd(wv[:, gsl]),
            "wo": _pack_kd(np.ascontiguousarray(wo[gsl, :])),
            "cosT": cosT,
            "sinT": sinT,
            "maskT": mask,
        })
    return in_maps


def assemble_output(results):
    """results: list of 8 dicts with 'out' [P, NKT, D] bf16 partials."""
    full = np.empty((B, S, D), dtype=np.float32)
    for b in range(B):
        acc = None
        for g in range(G):
            r = results[b * G + g]["out"].astype(np.float32)
            part = r.transpose(1, 0, 2).reshape(S, D)
            acc = part if acc is None else acc + part
        full[b] = acc
    return full


def _get_module():
    global _BUILT
    if _BUILT is None:
        _BUILT = build_module()
    return _BUILT


def _install_trace_shim():
    """This image's antenv lacks axon_hooks; provide the NTFF profile hook
    via ctypes so trace=True (or BASS_TRACE=1) works instead of crashing,
    and skip the artifact bucket upload."""
    try:
        import antenv.axon_hooks  # noqa: F401
        return
    except ImportError:
        pass
    import types
    import ctypes
    import contextlib

    so_path = "/opt/axon/libaxon_pjrt.so"
    mod = types.ModuleType("antenv.axon_hooks")
    try:
        lib = ctypes.CDLL(so_path)
        lib.axon_start_nrt_profile.argtypes = [
            ctypes.POINTER(ctypes.c_int64), ctypes.c_size_t]
        lib.axon_start_nrt_profile.restype = ctypes.c_int64
        lib.axon_stop_nrt_profile.argtypes = [ctypes.c_char_p]
        lib.axon_stop_nrt_profile.restype = ctypes.c_int64

        @contextlib.contextmanager
        def _hook(output_dir, device_ids):
            import jax
            jax.devices()
            if device_ids:
                ids = (ctypes.c_int64 * len(device_ids))(*device_ids)
                rc = lib.axon_start_nrt_profile(ids, len(device_ids))
            else:
                rc = lib.axon_start_nrt_profile(None, 0)
            if rc != 0:
                raise RuntimeError(f"axon_start_nrt_profile rc={rc}")
            try:
                yield
            finally:
                lib.axon_stop_nrt_profile(str(output_dir).encode())

        mod.get_axon_ntff_profile_hook = lambda: _hook
    except OSError:
        mod.get_axon_ntff_profile_hook = lambda: None
    mod.set_axon_ntff_profile_hook = lambda h: None
    sys.modules["antenv.axon_hooks"] = mod

    from concourse import bass_utils
    bass_utils.upload_artifacts = lambda tmpdir: tmpdir


def run_on_hw(in_maps, trace=False, trace_cores=None):
    _install_trace_shim()
    from concourse import bass_utils
    nc = _get_module()
    return bass_utils.run_bass_kernel_spmd(
        nc, in_maps, core_ids=list(range(N_CORES)),
        trace=trace, trace_cores=trace_cores)


def kernel(x, wq, wk, wv, wo):
    x = np.asarray(x, dtype=np.float32)
    wq = np.asarray(wq, dtype=np.float32)
    wk = np.asarray(wk, dtype=np.float32)
    wv = np.asarray(wv, dtype=np.float32)
    wo = np.asarray(wo, dtype=np.float32)
    in_maps = make_in_maps(x, wq, wk, wv, wo)
    res = run_on_hw(in_maps, trace=False)
    return assemble_output(res.results)



# revision 34
# speedup vs baseline: 1.0063x; 1.0008x over previous
"""Llama attention layer (B=2, S=2048, D=2048, H=16, HD=128, RoPE, causal)
on 8 Trainium2 NeuronCores.

Sharding: core c -> (batch b = c//4, head group g = c%4 of 4 heads).
Each core computes q/k/v projections for its 512 columns of wq/wk/wv,
RoPE, causal attention for its 4 heads, and the out-projection against
its 512 rows of wo (a partial sum over head groups). The host sums the
4 partials per batch and stacks the 2 batches.

All device matmuls run in bf16 with fp32 PSUM accumulation. Softmax is
computed without max-subtraction (scores here are bounded ~|9|), with
the denominator obtained from an M=1 ones-matmul over exp(scores^T).

Perf structure (vs the naive version; ~353us -> ~326us):
- xT is packed m-major on the host AND in SBUF so every DMA line is
  contiguous (strided 256B-line SBUF writes run ~7x slower); the
  K/Q-projection matmuls read it through a strided [128,4,128] rhs AP.
- Input DMAs are split over both hardware DGE queues (SP + Activation)
  because descriptor generation costs ~0.6us of sequencer time each.
- The V-projection runs k-outer over waves of 4 PSUM banks so each wv
  k-slice unlocks matmuls while the DMA trickles in; a few warmup
  matmuls on zeros pin the PE p-state early (the PE only reaches full
  clock after ~3us of continuous execution).
- Diagonal 128x512 score/exp/attn-V work is narrowed to the causally
  valid query columns; the V-matmul is split per 128-col region so each
  region's accumulation group can close with its own stop flag.
- Softmax denominator: DVE adds exp-chunk pairs, then pairs into quads;
  each quad's ones-matmul is deferred 4 chunks so the in-program-order
  PE never waits on the DVE adds. The last 4 chunks use immediate pair
  matmuls so the reciprocal isn't delayed.
- Attention for (qt=0, heads 0-1) runs at the tail of phase 1 on the 4
  free PSUM banks, hiding the phase-1 PSUM drain / phase-2 pool-open
  latency; each q-tile's out-projection is interleaved between the next
  tile's heads to hide the normalize latency.
- Output partials are written bf16 (host accumulates in fp32), halving
  output DMA.
"""

import os
import sys

import numpy as np
import ml_dtypes

if "/opt/trn_rl_repo" not in sys.path:
    sys.path.insert(0, "/opt/trn_rl_repo")

import concourse.bass as bass  # noqa: E402
import concourse.mybir as mybir  # noqa: E402
import concourse.bacc as bacc  # noqa: E402
import concourse.tile as tile  # noqa: E402

BF16 = ml_dtypes.bfloat16

B, S, D, H = 2, 2048, 2048, 16
HD = D // H            # 128, head dim
G = 4                  # head groups (cores per batch)
NH = H // G            # 4 heads per core
DG = NH * HD           # 512, per-core head width
P = 128
KO = D // P            # 16 k-subtiles over D
NKT = S // P           # 16 key chunks of 128
NQT = S // 512         # 4 q tiles of 512
QT = 512
ROPE_THETA = 10000.0
SCALE = 1.0 / float(np.sqrt(HD))

N_CORES = 8

_BUILT = None  # (nc,) cache


def build_module():
    fp32 = mybir.dt.float32
    bf16 = mybir.dt.bfloat16

    nc = bacc.Bacc("TRN2", target_bir_lowering=False, debug=False,
                   num_devices=N_CORES, num_swdge_queues=4)

    xT = nc.dram_tensor("xT", [P, NKT, KO, P], bf16, kind="ExternalInput")
    wq = nc.dram_tensor("wq", [P, KO, DG], bf16, kind="ExternalInput")
    wk = nc.dram_tensor("wk", [P, KO, DG], bf16, kind="ExternalInput")
    wv = nc.dram_tensor("wv", [P, KO, DG], bf16, kind="ExternalInput")
    wo = nc.dram_tensor("wo", [P, NH, D], bf16, kind="ExternalInput")
    cosT = nc.dram_tensor("cosT", [P, S], bf16, kind="ExternalInput")
    sinT = nc.dram_tensor("sinT", [P, S], bf16, kind="ExternalInput")
    maskT = nc.dram_tensor("maskT", [P, P], bf16, kind="ExternalInput")
    out = nc.dram_tensor("out", [P, NKT, D], bf16, kind="ExternalOutput")

    Exp = mybir.ActivationFunctionType.Exp

    with tile.TileContext(nc) as tc:
        with tc.tile_pool(name="const", bufs=1) as const, \
             tc.tile_pool(name="big", bufs=1) as big:
            ones = const.tile([P, P], bf16)
            nc.vector.memset(ones, 1.0)
            # dummy exp so the ACT Exp table loads during the DMA prefix,
            # not at the first real exp in the attention phase
            warm = const.tile([1, 1], fp32)
            nc.scalar.activation(warm, ones[0:1, 0:1],
                                 mybir.ActivationFunctionType.Exp)
            warm_mm = const.tile([P, QT], bf16)
            nc.vector.memset(warm_mm, 0.0)

            qT_sb = big.tile([P, NH, S], bf16)   # per head: [HD, S]
            kT_sb = big.tile([P, NH, S], bf16)
            v_sb = big.tile([P, NKT, DG], bf16)  # [key%128, keychunk, dg]
            wo_sb = big.tile([P, NH, D], bf16)
            mask_sb = const.tile([P, P], bf16)
            ao0_sb = big.tile([P, 2, QT], bf16)  # (qt=0, h=0/1) attention out

            def attend(qt, h, pool_s, s_bufs, pool_o, pool_sum, pool_ax,
                       ax_bufs, pool_axp, axp_bufs, pool_axq, pool_ep,
                       ep_bufs, dst):
                """Causal attention for one (q-tile, head) into dst."""
                n_kt = 4 * (qt + 1)  # causal: key chunks 0..n_kt-1
                ps_o = pool_o.tile([P, QT], fp32, tag="ps_o")
                # all-ones [128,128] lhsT -> every psum row holds sumexp:
                # no partition-broadcast needed later
                ps_sum = pool_sum.tile([P, QT], fp32, tag="ps_sum")
                # denominator plan: early chunk pairs are added into quads
                # on DVE and their ones-matmul is DEFERRED 4 chunks (so the
                # PE, which executes in program order, never waits on the
                # DVE adds); the last 4 chunks use immediate pair matmuls.
                n_equads = max(0, n_kt - 4) // 4
                mm_total = n_equads + 2
                mm_i = 0
                pending = {}
                ax_prev = None
                pair_prev = None
                for m in range(n_kt):
                    if m in pending:
                        nc.tensor.matmul(ps_sum, ones, pending.pop(m),
                                         start=(mm_i == 0),
                                         stop=(mm_i == mm_total - 1))
                        mm_i += 1
                    # diagonal chunks only cover queries >= their first key:
                    # narrow to columns [colo:QT)
                    o = m - qt * 4
                    colo = max(0, o) * P
                    ps_s = pool_s.tile([P, QT], fp32, tag="ps_s",
                                       bufs=s_bufs)
                    nc.tensor.matmul(ps_s[:, colo:],
                                     kT_sb[:, h, m * P:(m + 1) * P],
                                     qT_sb[:, h,
                                           qt * QT + colo:(qt + 1) * QT],
                                     start=True, stop=True)
                    ax = pool_ax.tile([P, QT], bf16, tag="ax", bufs=ax_bufs)
                    if colo:
                        # zero the causally-dead prefix so the denominator
                        # adds see zeros there
                        nc.gpsimd.memset(ax[:, 0:colo], 0.0)
                    nc.scalar.activation(ax[:, colo:], ps_s[:, colo:],
                                         Exp, scale=SCALE)
                    if o >= 0:
                        # triangular mask on the 128 cols that straddle the
                        # diagonal; later cols are fully valid
                        nc.vector.tensor_mul(ax[:, colo:colo + P],
                                             ax[:, colo:colo + P], mask_sb)
                    # attn @ V, narrowed. Each 128-col region's last
                    # contribution is the diagonal chunk o = region index,
                    # so that slice carries stop=True while the rest keeps
                    # accumulating.
                    vsl = v_sb[:, m, h * HD:(h + 1) * HD]
                    if o < 0:
                        nc.tensor.matmul(ps_o, vsl, ax,
                                         start=(m == 0), stop=False)
                    else:
                        nc.tensor.matmul(ps_o[:, colo:colo + P], vsl,
                                         ax[:, colo:colo + P],
                                         start=(m == 0), stop=True)
                        if colo + P < QT:
                            nc.tensor.matmul(ps_o[:, colo + P:], vsl,
                                             ax[:, colo + P:],
                                             start=(m == 0), stop=False)
                    if m % 2 == 0:
                        ax_prev = ax
                    else:
                        pair = pool_axp.tile([P, QT], bf16, tag="axp",
                                             bufs=axp_bufs)
                        nc.vector.tensor_add(pair, ax_prev, ax)
                        if m >= n_kt - 4:
                            nc.tensor.matmul(ps_sum, ones, pair,
                                             start=(mm_i == 0),
                                             stop=(mm_i == mm_total - 1))
                            mm_i += 1
                        elif m % 4 == 1:
                            pair_prev = pair
                        else:
                            quad = pool_axq.tile([P, QT], bf16, tag="axq",
                                                 bufs=3)
                            nc.vector.tensor_add(quad, pair_prev, pair)
                            pending[min(m + 4, n_kt - 1)] = quad
                rec = pool_ep.tile([P, QT], fp32, tag="rec", bufs=ep_bufs)
                nc.vector.reciprocal_approx_fast(rec, ps_sum)
                nc.vector.tensor_mul(dst, ps_o, rec)

            # ---------------- phase 1: projections + RoPE ----------------
            with tc.tile_pool(name="w_pool", bufs=1) as w_pool, \
                 tc.tile_pool(name="rope", bufs=4) as rope, \
                 tc.tile_pool(name="ps1", bufs=1, space="PSUM") as ps1:
                # DMA order matters: wv first (V-loop gate), then xT in
                # m-major column blocks (the dram layout is packed so block m
                # is contiguous) so V m-group m only waits for its own block,
                # then the K/Q-phase tensors, then phase-2/3 tensors.
                wv_sb = w_pool.tile([P, KO, DG], bf16)
                # m-major like the DRAM packing: per-block DMA is contiguous
                # (4KB/partition). A k-major SBUF layout would make the block
                # DMA scatter 256B lines, which runs ~7x slower.
                xT_sb = w_pool.tile([P, NKT, KO, P], bf16)
                wk_sb = w_pool.tile([P, KO, DG], bf16)
                cos_sb = w_pool.tile([P, S], bf16)
                sin_sb = w_pool.tile([P, S], bf16)
                wq_sb = w_pool.tile([P, KO, DG], bf16)
                # Descriptor generation costs ~0.6us of sequencer time per
                # dma_start, so split the input stream over BOTH hardware DGE
                # queues: the weight stream on the Activation queue (idle in
                # phase 1), the xT column blocks on the SP queue. The V-loop
                # m-group m then only waits for wv (k-sliced, so its first
                # matmuls start early) and its own xT block.
                # first k-slices as singles so the V-wave's first matmuls
                # start as early as possible, the rest coarser to keep the
                # per-dma descriptor-generation cost down
                for k in range(4):
                    nc.scalar.dma_start(wv_sb[:, k, :], wv.ap()[:, k, :])
                for ks_ in range(4, KO, 4):
                    nc.scalar.dma_start(wv_sb[:, ks_:ks_ + 4, :],
                                        wv.ap()[:, ks_:ks_ + 4, :])
                for m in range(NKT):
                    nc.sync.dma_start(xT_sb[:, m], xT.ap()[:, m])
                nc.scalar.dma_start(wk_sb, wk.ap())
                nc.scalar.dma_start(cos_sb, cosT.ap())
                nc.scalar.dma_start(sin_sb, sinT.ap())
                nc.scalar.dma_start(wq_sb, wq.ap())
                nc.sync.dma_start(mask_sb, maskT.ap())
                nc.sync.dma_start(wo_sb, wo.ap())

                # PE p-state warmup: the tensor engine clocks up only after
                # ~3us of continuous execution, so chew on zeros while the
                # first wv/xT DMAs land -- the first real matmuls then run at
                # full clock instead of half.
                ps_w = ps1.tile([P, QT], fp32, tag="psv", bufs=4)
                for r in range(5):
                    nc.tensor.matmul(ps_w, ones, warm_mm,
                                     start=(r == 0), stop=(r == 4))

                # V: [keys, dg] natural layout, keychunk tiles of 128.
                # k-OUTER waves of 4 m-groups (4 psum banks): each wv k-slice
                # unlocks 4 matmuls, so the PE ramps as the k-sliced wv DMA
                # trickles in instead of waiting for all of wv.
                for mw in range(0, NKT, 4):
                    pss = [ps1.tile([P, DG], fp32, tag="psv",
                                    name=f"psv{i}", bufs=4) for i in range(4)]
                    for k in range(KO):
                        for i in range(4):
                            nc.tensor.matmul(pss[i], xT_sb[:, mw + i, k, :],
                                             wv_sb[:, k, :],
                                             start=(k == 0),
                                             stop=(k == KO - 1))
                    for i in range(4):
                        nc.vector.tensor_copy(v_sb[:, mw + i, :], pss[i])

                # K then Q: [HD, S] transposed layout + RoPE.
                # Heads processed in pairs so the two psum tags can be
                # double-buffered (2 tags x 2 bufs) -- RoPE of one pair
                # overlaps the matmuls of the next.
                for which, w_sb, dstT in (("k", wk_sb, kT_sb), ("q", wq_sb, qT_sb)):
                    for nt2 in range(2 * NQT):
                        nt, hp = divmod(nt2, 2)
                        sl = slice(nt * QT, (nt + 1) * QT)
                        heads = (2 * hp, 2 * hp + 1)
                        # share the "psv" tag (banks 0-3) so all of phase 1
                        # stays within 4 psum banks, leaving 4-7 free for
                        # the attention pools to start without bank conflicts
                        pss = {}
                        for h in heads:
                            pss[h] = ps1.tile([P, QT], fp32, tag="psv",
                                              name=f"psp{h}", bufs=4)
                        # rhs: the nt-th 512 queries = xT m-blocks 4nt..4nt+3
                        # at fixed k -- a strided [128, 4, 128] AP
                        for k in range(KO):
                            for h in heads:
                                nc.tensor.matmul(
                                    pss[h], w_sb[:, k, h * HD:(h + 1) * HD],
                                    xT_sb[:, nt * 4:(nt + 1) * 4, k, :],
                                    start=(k == 0), stop=(k == KO - 1))
                        for h in heads:
                            ps = pss[h]
                            dst = dstT[:, h, sl]
                            # rope: dst = ps * cos + swap(ps) * sin_signed.
                            # The swapped reads must come from PSUM (the SB-SB
                            # same-base-partition rule forbids them on SBUF);
                            # the straight read goes via a parallel ACT copy so
                            # the psum bank drains fast.
                            tmp = rope.tile([P, QT], bf16, tag="tmp")
                            nc.vector.tensor_mul(tmp[0:64], ps[64:128],
                                                 sin_sb[0:64, sl])
                            nc.vector.tensor_mul(tmp[64:128], ps[0:64],
                                                 sin_sb[64:128, sl])
                            qb = rope.tile([P, QT], bf16, tag="qb")
                            nc.scalar.copy(qb, ps)
                            nc.vector.tensor_mul(dst, qb, cos_sb[:, sl])
                            nc.vector.tensor_add(dst, dst, tmp)

                # attention for (qt=0, h=0/1) right here: its inputs (kT/qT
                # tile 0, v) are long ready, and PSUM banks 4-7 are free
                # (phase 1 keeps to 4 banks via the shared psv tag). The PE
                # chews on these chunks while phase-1 PSUM drains and the
                # phase-2 pools open, bridging the transition gap.
                with tc.tile_pool(name="a0ps", bufs=1, space="PSUM") as a0ps, \
                     tc.tile_pool(name="a0sb", bufs=1) as a0sb:
                    for h0 in range(2):
                        attend(0, h0, a0ps, 2, a0ps, a0ps, a0sb, 4, a0sb, 2,
                               a0sb, a0sb, 1, ao0_sb[:, h0, :])

            # ---------------- phases 2+3 ----------------
            with tc.tile_pool(name="big2", bufs=1) as big2:
                aoT_sb = big2.tile([P, NH, S], bf16)  # attention out^T

                # phases 2+3 interleaved: attention for q-tile qt, then the
                # out-projection rows it unblocks (their matmuls have no ACT
                # dependency and fill the exp-latency bubbles)
                with tc.tile_pool(name="ax_pool", bufs=20) as ax_pool, \
                     tc.tile_pool(name="axp_pool", bufs=6) as axp_pool, \
                     tc.tile_pool(name="axq_pool", bufs=3) as axq_pool, \
                     tc.tile_pool(name="ep", bufs=4) as ep, \
                     tc.tile_pool(name="stage", bufs=6) as stage, \
                     tc.tile_pool(name="ps2s", bufs=3, space="PSUM") as ps2s, \
                     tc.tile_pool(name="ps2o", bufs=2, space="PSUM") as ps2o, \
                     tc.tile_pool(name="ps2", bufs=1, space="PSUM") as ps2, \
                     tc.tile_pool(name="ps3", bufs=2, space="PSUM") as ps3:
                    def emit_outproj(qo):
                        for n in range(D // QT):
                            nsl = slice(n * QT, (n + 1) * QT)
                            ps = ps3.tile([P, QT], fp32, tag="ps_out")
                            for h in range(NH):
                                if h < 2 and qo < 4:
                                    lhs = ao0_sb[:, h, qo * P:(qo + 1) * P]
                                else:
                                    lhs = aoT_sb[:, h, qo * P:(qo + 1) * P]
                                nc.tensor.matmul(
                                    ps, lhs, wo_sb[:, h, nsl],
                                    start=(h == 0), stop=(h == NH - 1))
                            ob = stage.tile([P, QT], bf16, tag="ob")
                            nc.vector.tensor_copy(ob, ps)
                            nc.sync.dma_start(out.ap()[:, qo, nsl], ob)

                    for qt in range(NQT):
                        qsl = slice(qt * QT, (qt + 1) * QT)
                        for h in range(NH):
                            if qt == 0 and h < 2:
                                continue  # computed at the end of phase 1
                            attend(qt, h, ps2s, 3, ps2o, ps2, ax_pool, 20,
                                   axp_pool, 6, axq_pool, ep, 4,
                                   aoT_sb[:, h, qsl])

                            # out-projection for the previous q-tile's rows,
                            # interleaved between this tile's heads so the
                            # normalize latency of the previous tile's last
                            # head hides under this head's chunk matmuls
                            if qt > 0:
                                emit_outproj(4 * (qt - 1) + h)

                    # the last q-tile's rows have nothing to hide behind
                    for qo in range(4 * (NQT - 1), 4 * NQT):
                        emit_outproj(qo)

    nc.compile()
    return nc


def _rope_tables():
    inv_freq = 1.0 / (ROPE_THETA ** (np.arange(0, HD, 2, dtype=np.float64) / HD))
    pos = np.arange(S, dtype=np.float64)
    freqs = np.outer(pos, inv_freq)                    # [S, HD/2]
    emb = np.concatenate([freqs, freqs], axis=-1)      # [S, HD]
    cos = np.cos(emb).T.astype(BF16)                   # [HD, S]
    sin = np.sin(emb).T.astype(np.float32)
    sin[: HD // 2] *= -1.0                             # fold rotate_half sign
    return cos, sin.astype(BF16)


def _pack_kd(a):
    """[D, N] -> [P, D//P, N] with d = ko*P + p."""
    d, n = a.shape
    return np.ascontiguousarray(
        a.reshape(d // P, P, n).transpose(1, 0, 2)).astype(BF16)


def _pack_xT(xb):
    """x[b] [S, D] -> [P, NKT, KO, P] m-major so each 128-col block of x^T
    is one contiguous DMA."""
    t = _pack_kd(np.ascontiguousarray(xb.T))           # [P, KO, S]
    return np.ascontiguousarray(
        t.reshape(P, KO, NKT, P).transpose(0, 2, 1, 3))


def make_in_maps(x, wq, wk, wv, wo):
    cosT, sinT = _rope_tables()
    i = np.arange(P)[:, None]
    j = np.arange(P)[None, :]
    mask = (i <= j).astype(BF16)

    xT_packed = [_pack_xT(x[b]) for b in range(B)]
    in_maps = []
    for c in range(N_CORES):
        b, g = divmod(c, G)
        gsl = slice(g * DG, (g + 1) * DG)
        in_maps.append({
            "xT": xT_packed[b],
            "wq": _pack_kd(wq[:, gsl]),
            "wk": _pack_kd(wk[:, gsl]),
            "wv": _pack_kd(wv[:, gsl]),
            "wo": _pack_kd(np.ascontiguousarray(wo[gsl, :])),
            "cosT": cosT,
            "sinT": sinT,
            "maskT": mask,
        })
    return in_maps


def assemble_output(results):
    """results: list of 8 dicts with 'out' [P, NKT, D] bf16 partials."""
    full = np.empty((B, S, D), dtype=np.float32)
    for b in range(B):
        acc = None
        for g in range(G):
            r = results[b * G + g]["out"].astype(np.float32)
            part = r.transpose(1, 0, 2).reshape(S, D)
            acc = part if acc is None else acc + part
        full[b] = acc
    return full


def _get_module():
    global _BUILT
    if _BUILT is None:
        _BUILT = build_module()
    return _BUILT


def _install_trace_shim():
    """This image's antenv lacks axon_hooks; provide the NTFF profile hook
    via ctypes so trace=True (or BASS_TRACE=1) works instead of crashing,
    and skip the artifact bucket upload."""
    try:
        import antenv.axon_hooks  # noqa: F401
        return
    except ImportError:
        pass
    import types
    import ctypes
    import contextlib

    so_path = "/opt/axon/libaxon_pjrt.so"
    mod = types.ModuleType("antenv.axon_hooks")
    try:
        lib = ctypes.CDLL(so_path)
        lib.axon_start_nrt_profile.argtypes = [
            ctypes.POINTER(ctypes.c_int64), ctypes.c_size_t]
        lib.axon_start_nrt_profile.restype = ctypes.c_int64
        lib.axon_stop_nrt_profile.argtypes = [ctypes.c_char_p]
        lib.axon_stop_nrt_profile.restype = ctypes.c_int64

        @contextlib.contextmanager
        def _hook(output_dir, device_ids):
            import jax
            jax.devices()
            if device_ids:
                ids = (ctypes.c_int64 * len(device_ids))(*device_ids)
                rc = lib.axon_start_nrt_profile(ids, len(device_ids))
            else:
                rc = lib.axon_start_nrt_profile(None, 0)
            if rc != 0:
                raise RuntimeError(f"axon_start_nrt_profile rc={rc}")
            try:
                yield
            finally:
                lib.axon_stop_nrt_profile(str(output_dir).encode())

        mod.get_axon_ntff_profile_hook = lambda: _hook
    except OSError:
        mod.get_axon_ntff_profile_hook = lambda: None
    mod.set_axon_ntff_profile_hook = lambda h: None
    sys.modules["antenv.axon_hooks"] = mod

    from concourse import bass_utils
    bass_utils.upload_artifacts = lambda tmpdir: tmpdir


def run_on_hw(in_maps, trace=False, trace_cores=None):
    _install_trace_shim()
    from concourse import bass_utils
    nc = _get_module()
    return bass_utils.run_bass_kernel_spmd(
        nc, in_maps, core_ids=list(range(N_CORES)),
        trace=trace, trace_cores=trace_cores)


def kernel(x, wq, wk, wv, wo):
    x = np.asarray(x, dtype=np.float32)
    wq = np.asarray(wq, dtype=np.float32)
    wk = np.asarray(wk, dtype=np.float32)
    wv = np.asarray(wv, dtype=np.float32)
    wo = np.asarray(wo, dtype=np.float32)
    in_maps = make_in_maps(x, wq, wk, wv, wo)
    res = run_on_hw(in_maps, trace=False)
    return assemble_output(res.results)

